# revision 2
# baseline (speedup 1.0000x reference)
"""Trainium2 Bass kernel for nn_BallModel: 10M-step ballistic trajectory.

The reference recurrence (pos += vel*dt; vel += g*dt, recording pos) has the
closed form
    pos_i = pos0 + i*dt*vel0 + g*dt^2 * i*(i-1)/2  =  A + B*i + C*i^2
with A = pos0, B = dt*vel0 - C, C = (g*dt)*dt/2 (per component; C_x = 0).

Output is [10_000_000, 2] f32 (~80 MB), interleaved x,y.  Each of the 8 cores
produces a contiguous 2.5M-element slice (10 MB) -> memory-bound at the
per-core HBM write bandwidth (~380 GB/s measured => ~26.3 us drain floor).

Layout (v2, "contiguous-per-partition groups"): a core's slice is 9 full
groups + 1 tail group.  Within group g, partition p owns a CONTIGUOUS run of
JSPAN=1024 pairs (2048 f32) of the trajectory:

    pair index i = core*1_250_000 + g*131072 + p*1024 + j,   j = ce>>1

so one group is a [128, 2048] f32 tile whose DRAM image is a contiguous 1 MB
block with 8 KB contiguous per partition -> cheap HWDGE descriptors (128 x
8 KB per DMA instead of 512 x 2 KB).  The tail group covers the leftover
70352 pairs as [128, 1100] (550 pairs per partition, 48 pairs overshoot into
this core's own DRAM padding, trimmed on the host).

Each group is produced by 4 matmuls (N=512 each) that share ONE stationary
lhsT [K, 128] into one 4-bank PSUM tile [128, 2048]; with element q =
q(core,g,p) per-partition and j per-column:

    out[p, ce] = even*basex(q) + odd*(basey(q) + s1(q)*j) + resid(ce)
    basex = A_x + B_x q;  basey = A_y + B_y q + C q^2;  s1 = B_y + 2 C q
    resid = B_x*j on even cols, C*j^2 on odd cols

All values are bf16-split (2-3 parts) so products accumulate near-exactly in
the fp32 PSUM accumulator; j (up to 1023) is split j = 256a + b so the j
rows stay exact in bf16.  K=15 rows total; matmul cost only scales with N.

Pipeline per group: 4 MM -> one whole-tile PSUM->SBUF copy on a single
engine (even groups: scalar/ACT, odd groups: vector/DVE, each paired with
its own 4-bank PSUM pool so WAR chains stay single-semaphore) -> one 1 MB
HWDGE DMA.  One copy engine per group (instead of a split) keeps every DMA
at a single wait and roughly halves the program's dependency-edge count vs
the per-512-col-chunk baseline: each edge costs a hardware event semaphore,
and the NEFF epilogue resets them ONE INSTRUCTION AT A TIME (~25-125 ns
each, serialized per engine) AFTER the last DMA completes — the baseline
burned ~7.7 us there (254 sems).

Structural notes:
 - built on bacc.Bacc, NOT raw bass.Bass, so that legalization runs;
 - every group gets its own SBUF output tile (10 MB of SBUF) so copies
   carry no WAR waits on earlier output DMAs;
 - all DMAs on the sync HWDGE path (gpsimd SWDGE stalls; scalar HWDGE
   hard-hung the device when tried for input loads).
"""

import sys
import types

import ml_dtypes
import numpy as np

import concourse.bacc as bacc
import concourse.bass as bass
import concourse.mybir as mybir
from concourse.bass_utils import run_bass_kernel_spmd
from concourse.tile import TileContext

# ---- problem constants (hardcoded; kernel.py must be self-contained) ----
N_PAIRS = 10_000_000
N_CORES = 8
CP = N_PAIRS // N_CORES  # 1,250,000 pairs per core
P = 128  # partitions
JSPAN = 1024  # pairs per partition per full group
GCOLS = 2 * JSPAN  # 2048 f32 per partition per full group
GPAIRS = P * JSPAN  # 131072 pairs per full group
NGF = CP // GPAIRS  # 9 full groups
TPAIRS = CP - NGF * GPAIRS  # 70352 tail pairs
TJSPAN = -(-TPAIRS // P)  # 550 pairs per partition in the tail group
TCOLS = 2 * TJSPAN  # 1100 f32 columns in the tail group
NG = NGF + 1  # 10 groups
K = 15  # matmul contraction rows

# fp32-rounded constants, matching the reference's fp32 parameter rounding
DT = float(np.float32(0.01))
GDT_Y = float(np.float32(np.float32(-9.81) * np.float32(0.01)))  # fp32(g_y*dt)
C_Y = GDT_Y * DT / 2.0  # i^2 coefficient for y

_bf16 = ml_dtypes.bfloat16

# exposed for test.py introspection (exec_time_ns etc.)
LAST_RESULTS = None


def _ensure_axon_hooks_stub():
    """bass_utils imports antenv.axon_hooks when BASS_TRACE is set; some
    images lack that module.  Register a stub that degrades to the untraced
    path instead of crashing (test.py replaces it with a real NTFF hook)."""
    try:
        import antenv.axon_hooks  # noqa: F401

        return
    except ImportError:
        pass
    try:
        import antenv  # noqa: F401
    except ImportError:
        return
    stub = types.ModuleType("antenv.axon_hooks")
    stub.get_axon_ntff_profile_hook = lambda: None
    stub.set_axon_ntff_profile_hook = lambda h: None
    sys.modules["antenv.axon_hooks"] = stub


def _group_cols(g):
    return GCOLS if g < NGF else TCOLS


def _build_program() -> bass.Bass:
    # Bacc (not raw Bass): its finalize pipeline runs the sync-wait
    # legalization and register allocation walrus requires.
    nc = bacc.Bacc("TRN2", target_bir_lowering=False)
    # One "head" input carries rh + group 0's lhsT so a single DMA gates the
    # first matmul; the remaining groups' lhsT loads concurrently behind it.
    hd = nc.declare_dram_parameter(
        "hd", [K, GCOLS + P], mybir.dt.bfloat16, isOutput=False
    )
    lt_t = nc.declare_dram_parameter(
        "lt_t", [K, (NG - 1) * P], mybir.dt.bfloat16, isOutput=False
    )
    out = nc.declare_dram_parameter(
        "out", [NG * P, GCOLS], mybir.dt.float32, isOutput=True
    )

    with TileContext(nc) as tc:
        with (
            tc.tile_pool(name="const", bufs=1) as cpool,
            tc.tile_pool(name="work", bufs=1) as wpool,
            tc.tile_pool(name="psum_a", bufs=1, space="PSUM") as ppool_a,
            tc.tile_pool(name="psum_b", bufs=1, space="PSUM") as ppool_b,
        ):
            hd_s = cpool.tile([K, GCOLS + P], mybir.dt.bfloat16)
            ltt_s = cpool.tile([K, (NG - 1) * P], mybir.dt.bfloat16)
            nc.sync.dma_start(hd_s[:], hd[:])
            nc.sync.dma_start(ltt_s[:], lt_t[:])
            rh_s = hd_s[:, :GCOLS]

            def lhsT(g):
                if g == 0:
                    return hd_s[:, GCOLS : GCOLS + P]
                return ltt_s[:, (g - 1) * P : g * P]

            for g in range(NG):
                cols = _group_cols(g)
                nmm = -(-cols // 512)
                pool = ppool_a if g % 2 == 0 else ppool_b
                pt = pool.tile([P, GCOLS], mybir.dt.float32, name=f"pt{g % 2}",
                               tag=f"pt{g % 2}")
                lt = lhsT(g)
                for m in range(nmm):
                    c0 = m * 512
                    c1 = min(c0 + 512, cols)
                    nc.tensor.matmul(
                        pt[:, c0:c1], lt, rh_s[:, c0:c1], start=True, stop=True
                    )
                ot = wpool.tile([P, cols], mybir.dt.float32, name=f"ot{g}",
                                tag=f"ot{g}")
                if g % 2 == 0:
                    nc.scalar.copy(ot[:], pt[:, :cols])
                else:
                    nc.vector.tensor_copy(ot[:], pt[:, :cols])
                nc.sync.dma_start(out[g * P : (g + 1) * P, :cols], ot[:])
    nc.finalize()  # runs Bacc.compile(): reg alloc + sync-wait legalization
    return nc


def _split_bf16(x: np.ndarray, n: int):
    """Split x into n bf16 parts summing (nearly) exactly to x."""
    parts = []
    rem = np.asarray(x, dtype=np.float64).copy()
    for _ in range(n):
        p = rem.astype(_bf16)
        parts.append(p)
        rem = rem - p.astype(np.float64)
    return parts


def _host_tables(pos0: np.ndarray, vel0: np.ndarray):
    """Build per-core input tables (float64 math, cast at the end)."""
    ax, ay = float(pos0[0]), float(pos0[1])
    bx_c = DT * float(vel0[0])  # B_x (C_x = 0)
    by_c = DT * float(vel0[1]) - C_Y  # B_y

    # fixed rhs column patterns over ce in [0, GCOLS)
    ce = np.arange(GCOLS)
    j = (ce >> 1).astype(np.float64)
    odd = (ce & 1).astype(np.float64)
    even = 1.0 - odd
    ja = (256.0 * np.floor(j / 256.0)) * odd  # multiples of 256: exact bf16
    jb = (j - 256.0 * np.floor(j / 256.0)) * odd  # 0..255: exact bf16
    resid = np.where(ce & 1 == 1, C_Y * j * j, bx_c * j)
    r3 = _split_bf16(resid, 3)
    oddb = odd.astype(_bf16)
    evenb = even.astype(_bf16)
    rh_np = np.stack(
        [ja.astype(_bf16)] * 3
        + [jb.astype(_bf16)] * 3
        + r3
        + [oddb] * 3
        + [evenb] * 3
    )  # [K, GCOLS]

    in_maps = []
    p_idx = np.arange(P, dtype=np.float64)
    for k in range(N_CORES):
        lt_cols = []
        for g in range(NG):
            span = JSPAN if g < NGF else TJSPAN
            q = k * CP + g * GPAIRS + p_idx * span  # [P]
            s1_3 = _split_bf16(by_c + 2.0 * C_Y * q, 3)
            by3 = _split_bf16(ay + by_c * q + C_Y * q * q, 3)
            bx3 = _split_bf16(ax + bx_c * q, 3)
            ones = np.ones_like(s1_3[0])
            rows = s1_3 + s1_3 + [ones] * 3 + by3 + bx3
            lt_cols.append(np.stack(rows))  # [K, P]
        lt_np = np.concatenate(lt_cols, axis=1)  # [K, NG*P]
        in_maps.append(
            {
                "hd": np.ascontiguousarray(
                    np.concatenate([rh_np, lt_np[:, :P]], axis=1)
                ),
                "lt_t": np.ascontiguousarray(lt_np[:, P:]),
            }
        )
    return in_maps


def kernel(ball_mass, ball_initial_position, ball_initial_velocity) -> np.ndarray:
    global LAST_RESULTS
    pos0 = np.asarray(ball_initial_position, dtype=np.float32)
    vel0 = np.asarray(ball_initial_velocity, dtype=np.float32)

    _ensure_axon_hooks_stub()
    nc = _build_program()
    in_maps = _host_tables(pos0, vel0)
    res = run_bass_kernel_spmd(nc, in_maps, core_ids=list(range(N_CORES)))
    LAST_RESULTS = res

    parts = []
    for r in res.results:
        arr = np.asarray(r["out"], dtype=np.float32)  # [NG*P, GCOLS]
        main = arr[: NGF * P, :].reshape(-1)  # 9 full groups, contiguous
        tail = arr[NGF * P :, :TCOLS].reshape(-1)[: 2 * TPAIRS]
        parts.append(main)
        parts.append(tail)
    return np.concatenate(parts).reshape(N_PAIRS, 2)


if __name__ == "__main__":
    import os

    pos0 = (
        np.load("/tmp/pos0.npy")
        if os.path.exists("/tmp/pos0.npy")
        else np.array([-1.866805, -0.25733662], np.float32)
    )
    vel0 = (
        np.load("/tmp/vel0.npy")
        if os.path.exists("/tmp/vel0.npy")
        else np.array([-0.847358, -1.5444987], np.float32)
    )
    outv = kernel(np.ones(()), pos0, vel0)
    i = np.arange(N_PAIRS, dtype=np.float64)[:, None]
    closed = (
        pos0.astype(np.float64)
        + i * DT * vel0.astype(np.float64)
        + np.array([0.0, GDT_Y * DT]) * i * (i - 1) / 2.0
    )
    err = np.abs(outv - closed)
    denom = np.maximum(np.abs(closed), 1e-12)
    print("closed-form maxabs-ratio rel err:", err.max() / np.abs(closed).max())
    print("closed-form max elementwise rel err:", (err / denom).max())


# revision 4
# speedup vs baseline: 1.0314x; 1.0314x over previous
"""Trainium2 Bass kernel for nn_BallModel: 10M-step ballistic trajectory.

The reference recurrence (pos += vel*dt; vel += g*dt, recording pos) has the
closed form
    pos_i = pos0 + i*dt*vel0 + g*dt^2 * i*(i-1)/2  =  A + B*i + C*i^2
with A = pos0, B = dt*vel0 - C, C = (g*dt)*dt/2 (per component; C_x = 0).

Output is [10_000_000, 2] f32 (~80 MB), interleaved x,y.  Each of the 8 cores
produces a contiguous 2.5M-element slice (10 MB) -> memory-bound at the
per-core HBM write bandwidth (~380-440 GB/s measured => ~24-26 us drain).

Layout ("contiguous-per-partition groups"): a core's slice is 4 ramp chunks
(one group-equivalent), 9 full groups, and 1 tail group.  Within full group
g, partition p owns a CONTIGUOUS run of JSPAN=1024 pairs (2048 f32):

    pair index i = core*1_250_000 + g*131072 + p*1024 + j,   j = ce>>1

so one group is a [128, 2048] f32 tile whose DRAM image is a contiguous 1 MB
block with 8 KB contiguous per partition -> cheap HWDGE descriptors and a
~420 GB/s drain (vs ~380 with 2 KB descriptors).  The ramp chunks are
[128, 512] with 256-pair spans (fast first copy/DMA to start the drain
early); the tail covers the leftover 70352 pairs as [128, 1100] (550 pairs
per partition) into its own contiguous DRAM tensor, scheduled EARLY so the
drain never ends on a strided straggler.

Each group is produced by matmuls (N<=512 each) that share ONE stationary
lhsT [K, 128] into a 4-bank PSUM tile; with pair index q = q(core,g,p)
per-partition and j per-column:

    out[p, ce] = even*basex(q) + odd*(basey(q) + s1(q)*j) + resid(ce)
    basex = A_x + B_x q;  basey = A_y + B_y q + C q^2;  s1 = B_y + 2 C q
    resid = B_x*j on even cols, C*j^2 on odd cols

All values are bf16-split (3 parts) so products accumulate near-exactly in
the fp32 PSUM accumulator; j (up to 1023) is split j = 256a + b so the j
rows stay exact in bf16.  K=15 rows; matmul cost only scales with N.

Pipeline per group: MMs -> one whole-tile PSUM->SBUF copy on a single
engine (alternating scalar/ACT and vector/DVE, each paired with its own
4-bank PSUM pool so WAR chains stay single-semaphore) -> one HWDGE DMA.
Single-engine copies keep every DMA at one wait and halve the program's
dependency-edge count: each edge costs a hardware event semaphore that the
NEFF epilogue resets ONE INSTRUCTION AT A TIME after the last DMA completes.

Structural notes:
 - built on bacc.Bacc, NOT raw bass.Bass, so that legalization runs;
 - every group gets its own SBUF output tile (10 MB of SBUF) so copies
   carry no WAR waits on earlier output DMAs;
 - all DMAs on the sync HWDGE path (gpsimd SWDGE stalls; scalar HWDGE
   hard-hung the device when tried for input loads).
"""

import sys
import types

import ml_dtypes
import numpy as np

import concourse.bacc as bacc
import concourse.bass as bass
import concourse.mybir as mybir
from concourse.bass_utils import run_bass_kernel_spmd
from concourse.tile import TileContext

# ---- problem constants (hardcoded; kernel.py must be self-contained) ----
N_PAIRS = 10_000_000
N_CORES = 8
CP = N_PAIRS // N_CORES  # 1,250,000 pairs per core
P = 128  # partitions
JSPAN = 1024  # pairs per partition per full group
GCOLS = 2 * JSPAN  # 2048 f32 per partition per full group
GPAIRS = P * JSPAN  # 131072 pairs per full group
NRAMP = 4  # leading [128,512] chunks (= one group-equivalent)
RSPAN = 256  # pairs per partition per ramp chunk
RPAIRS = P * RSPAN  # 32768 pairs per ramp chunk
NGF = CP // GPAIRS  # 9 full big groups (group 0 is the ramp)
TPAIRS = CP - NGF * GPAIRS  # 70352 tail pairs
TJSPAN = -(-TPAIRS // P)  # 550 pairs per partition in the tail group
TCOLS = 2 * TJSPAN  # 1100 f32 columns in the tail group
K = 15  # matmul contraction rows

# fp32-rounded constants, matching the reference's fp32 parameter rounding
DT = float(np.float32(0.01))
GDT_Y = float(np.float32(np.float32(-9.81) * np.float32(0.01)))  # fp32(g_y*dt)
C_Y = GDT_Y * DT / 2.0  # i^2 coefficient for y

_bf16 = ml_dtypes.bfloat16

# exposed for test.py introspection (exec_time_ns etc.)
LAST_RESULTS = None


def _ensure_axon_hooks_stub():
    """bass_utils imports antenv.axon_hooks when BASS_TRACE is set; some
    images lack that module.  Register a stub that degrades to the untraced
    path instead of crashing (test.py replaces it with a real NTFF hook)."""
    try:
        import antenv.axon_hooks  # noqa: F401

        return
    except ImportError:
        pass
    try:
        import antenv  # noqa: F401
    except ImportError:
        return
    stub = types.ModuleType("antenv.axon_hooks")
    stub.get_axon_ntff_profile_hook = lambda: None
    stub.set_axon_ntff_profile_hook = lambda h: None
    sys.modules["antenv.axon_hooks"] = stub


# host-side lhsT column order: ramp 0-3, tail, big groups 1-9
N_LT = NRAMP + 1 + (NGF - 1)  # 14 lhsT blocks
HD0_COLS = 512 + (NRAMP + 1) * P  # rh[:, :512] + ramp lhsT + tail lhsT
HD1_COLS = GCOLS - 512  # rh[:, 512:]
LT_COLS = (NGF - 1) * P  # big groups 1-9 lhsT


def _build_program() -> bass.Bass:
    # Bacc (not raw Bass): its finalize pipeline runs the sync-wait
    # legalization and register allocation walrus requires.
    nc = bacc.Bacc("TRN2", target_bir_lowering=False)
    # hd0 carries what the ramp chunks and the tail group need, so one small
    # DMA gates the first matmul; the rest loads concurrently behind it.
    hd0 = nc.declare_dram_parameter(
        "hd0", [K, HD0_COLS], mybir.dt.bfloat16, isOutput=False
    )
    hd1 = nc.declare_dram_parameter(
        "hd1", [K, HD1_COLS], mybir.dt.bfloat16, isOutput=False
    )
    lt_t = nc.declare_dram_parameter(
        "lt_t", [K, LT_COLS], mybir.dt.bfloat16, isOutput=False
    )
    out = nc.declare_dram_parameter(
        "out", [NGF * P, GCOLS], mybir.dt.float32, isOutput=True
    )
    outt = nc.declare_dram_parameter(
        "outt", [P, TCOLS], mybir.dt.float32, isOutput=True
    )

    with TileContext(nc) as tc:
        with (
            tc.tile_pool(name="const", bufs=1) as cpool,
            tc.tile_pool(name="work", bufs=1) as wpool,
            tc.tile_pool(name="psum_a", bufs=1, space="PSUM") as ppool_a,
            tc.tile_pool(name="psum_b", bufs=1, space="PSUM") as ppool_b,
        ):
            rh_s = cpool.tile([K, GCOLS], mybir.dt.bfloat16)
            aux_s = cpool.tile([K, (NRAMP + 1) * P], mybir.dt.bfloat16)
            ltt_s = cpool.tile([K, LT_COLS], mybir.dt.bfloat16)
            hd0_s_parts = (rh_s[:, :512], aux_s[:])
            # one DMA fills rh[:, :512] and the ramp+tail lhsT (two dsts
            # would need two DMAs; instead view hd0 as two slices)
            nc.sync.dma_start(hd0_s_parts[0], hd0[:, :512])
            nc.sync.dma_start(hd0_s_parts[1], hd0[:, 512:])
            nc.sync.dma_start(rh_s[:, 512:], hd1[:])
            nc.sync.dma_start(ltt_s[:], lt_t[:])

            def lhsT(idx):
                # idx: 0..NRAMP-1 ramp, NRAMP tail, NRAMP+1.. big groups 1-9
                if idx <= NRAMP:
                    return aux_s[:, idx * P : (idx + 1) * P]
                i = idx - NRAMP - 1
                return ltt_s[:, i * P : (i + 1) * P]

            pools = (ppool_a, ppool_b)
            copies = (nc.scalar.copy, nc.vector.tensor_copy)
            unit = 0  # alternator over (pool, copy engine)

            def produce(lt, cols, dst, name):
                nonlocal unit
                u = unit % 2
                unit += 1
                pt = pools[u].tile(
                    [P, GCOLS], mybir.dt.float32, name=f"pt{u}", tag=f"pt{u}"
                )
                for c0 in range(0, cols, 512):
                    c1 = min(c0 + 512, cols)
                    nc.tensor.matmul(
                        pt[:, c0:c1], lt, rh_s[:, c0:c1], start=True, stop=True
                    )
                ot = wpool.tile([P, cols], mybir.dt.float32, name=name, tag=name)
                copies[u](ot[:], pt[:, :cols])
                nc.sync.dma_start(dst, ot[:])

            # ramp chunks: pairs [c*RPAIRS, (c+1)*RPAIRS), 256-pair spans.
            # dst rows: out[32c:32c+32, :] viewed [32,4,512] -> partition
            # p = r*4+s owns the contiguous 2 KB at (r, s*512).
            for c in range(NRAMP):
                dst = out[c * 32 : (c + 1) * 32, :].rearrange(
                    "r (s q) -> (r s) q", s=4
                )
                produce(lhsT(c), 512, dst, f"or{c}")
            # tail group early (own contiguous tensor)
            produce(lhsT(NRAMP), TCOLS, outt[:], "ott")
            # big groups 1-9
            for g in range(1, NGF):
                produce(
                    lhsT(NRAMP + g), GCOLS, out[g * P : (g + 1) * P, :], f"og{g}"
                )
    nc.finalize()  # runs Bacc.compile(): reg alloc + sync-wait legalization
    return nc


def _split_bf16(x: np.ndarray, n: int):
    """Split x into n bf16 parts summing (nearly) exactly to x."""
    parts = []
    rem = np.asarray(x, dtype=np.float64).copy()
    for _ in range(n):
        p = rem.astype(_bf16)
        parts.append(p)
        rem = rem - p.astype(np.float64)
    return parts


def _host_tables(pos0: np.ndarray, vel0: np.ndarray):
    """Build per-core input tables (float64 math, cast at the end)."""
    ax, ay = float(pos0[0]), float(pos0[1])
    bx_c = DT * float(vel0[0])  # B_x (C_x = 0)
    by_c = DT * float(vel0[1]) - C_Y  # B_y

    # fixed rhs column patterns over ce in [0, GCOLS)
    ce = np.arange(GCOLS)
    j = (ce >> 1).astype(np.float64)
    odd = (ce & 1).astype(np.float64)
    even = 1.0 - odd
    ja = (256.0 * np.floor(j / 256.0)) * odd  # multiples of 256: exact bf16
    jb = (j - 256.0 * np.floor(j / 256.0)) * odd  # 0..255: exact bf16
    resid = np.where(ce & 1 == 1, C_Y * j * j, bx_c * j)
    r3 = _split_bf16(resid, 3)
    oddb = odd.astype(_bf16)
    evenb = even.astype(_bf16)
    rh_np = np.stack(
        [ja.astype(_bf16)] * 3
        + [jb.astype(_bf16)] * 3
        + r3
        + [oddb] * 3
        + [evenb] * 3
    )  # [K, GCOLS]

    def lt_block(q):  # q: [P] start pair index per partition
        s1_3 = _split_bf16(by_c + 2.0 * C_Y * q, 3)
        by3 = _split_bf16(ay + by_c * q + C_Y * q * q, 3)
        bx3 = _split_bf16(ax + bx_c * q, 3)
        ones = np.ones_like(s1_3[0])
        return np.stack(s1_3 + s1_3 + [ones] * 3 + by3 + bx3)  # [K, P]

    in_maps = []
    p_idx = np.arange(P, dtype=np.float64)
    for k in range(N_CORES):
        base = float(k * CP)
        blocks = []
        for c in range(NRAMP):  # ramp chunks
            blocks.append(lt_block(base + c * RPAIRS + p_idx * RSPAN))
        blocks.append(lt_block(base + NGF * GPAIRS + p_idx * TJSPAN))  # tail
        for g in range(1, NGF):  # big groups
            blocks.append(lt_block(base + g * GPAIRS + p_idx * JSPAN))
        lt_np = np.concatenate(blocks, axis=1)  # [K, N_LT*P]
        n_aux = (NRAMP + 1) * P
        in_maps.append(
            {
                "hd0": np.ascontiguousarray(
                    np.concatenate([rh_np[:, :512], lt_np[:, :n_aux]], axis=1)
                ),
                "hd1": np.ascontiguousarray(rh_np[:, 512:]),
                "lt_t": np.ascontiguousarray(lt_np[:, n_aux:]),
            }
        )
    return in_maps


def kernel(ball_mass, ball_initial_position, ball_initial_velocity) -> np.ndarray:
    global LAST_RESULTS
    pos0 = np.asarray(ball_initial_position, dtype=np.float32)
    vel0 = np.asarray(ball_initial_velocity, dtype=np.float32)

    _ensure_axon_hooks_stub()
    nc = _build_program()
    in_maps = _host_tables(pos0, vel0)
    res = run_bass_kernel_spmd(nc, in_maps, core_ids=list(range(N_CORES)))
    LAST_RESULTS = res

    parts = []
    for r in res.results:
        arr = np.asarray(r["out"], dtype=np.float32)  # [NGF*P, GCOLS]
        tail = np.asarray(r["outt"], dtype=np.float32)  # [P, TCOLS]
        parts.append(arr.reshape(-1))  # ramp + big groups, contiguous
        parts.append(tail.reshape(-1)[: 2 * TPAIRS])
    return np.concatenate(parts).reshape(N_PAIRS, 2)


if __name__ == "__main__":
    import os

    pos0 = (
        np.load("/tmp/pos0.npy")
        if os.path.exists("/tmp/pos0.npy")
        else np.array([-1.866805, -0.25733662], np.float32)
    )
    vel0 = (
        np.load("/tmp/vel0.npy")
        if os.path.exists("/tmp/vel0.npy")
        else np.array([-0.847358, -1.5444987], np.float32)
    )
    outv = kernel(np.ones(()), pos0, vel0)
    i = np.arange(N_PAIRS, dtype=np.float64)[:, None]
    closed = (
        pos0.astype(np.float64)
        + i * DT * vel0.astype(np.float64)
        + np.array([0.0, GDT_Y * DT]) * i * (i - 1) / 2.0
    )
    err = np.abs(outv - closed)
    denom = np.maximum(np.abs(closed), 1e-12)
    print("closed-form maxabs-ratio rel err:", err.max() / np.abs(closed).max())
    print("closed-form max elementwise rel err:", (err / denom).max())


# revision 9
# speedup vs baseline: 1.1516x; 1.1165x over previous
"""Trainium2 Bass kernel for nn_BallModel: 10M-step ballistic trajectory.

The reference recurrence (pos += vel*dt; vel += g*dt, recording pos) has the
closed form
    pos_i = pos0 + i*dt*vel0 + g*dt^2 * i*(i-1)/2  =  A + B*i + C*i^2
with A = pos0, B = dt*vel0 - C, C = (g*dt)*dt/2 (per component; C_x = 0).

Output is [10_000_000, 2] f32 (~80 MB), interleaved x,y.  Each of the 8 cores
produces a contiguous 2.5M-element slice (10 MB) -> memory-bound at the
per-core HBM write bandwidth (~380-440 GB/s measured => ~24-26 us drain).

Layout ("contiguous-per-partition groups"): a core's slice is 4 ramp chunks
(one group-equivalent), 9 full groups, and 1 tail group.  Within full group
g, partition p owns a CONTIGUOUS run of JSPAN=1024 pairs (2048 f32):

    pair index i = core*1_250_000 + g*131072 + p*1024 + j,   j = ce>>1

so one group is a [128, 2048] f32 tile whose DRAM image is a contiguous 1 MB
block with 8 KB contiguous per partition -> cheap HWDGE descriptors and a
~420 GB/s drain (vs ~380 with 2 KB descriptors).  The ramp chunks are
[128, 512] with 256-pair spans (fast first copy/DMA to start the drain
early); the tail covers the leftover 70352 pairs as [128, 1100] (550 pairs
per partition) into its own contiguous DRAM tensor, scheduled EARLY so the
drain never ends on a strided straggler.

Each group is produced by matmuls (N<=512 each) that share ONE stationary
lhsT [K, 128] into a 4-bank PSUM tile; with pair index q = q(core,g,p)
per-partition and j per-column:

    out[p, ce] = even*basex(q) + odd*(basey(q) + s1(q)*j) + resid(ce)
    basex = A_x + B_x q;  basey = A_y + B_y q + C q^2;  s1 = B_y + 2 C q
    resid = B_x*j on even cols, C*j^2 on odd cols

All values are bf16-split (3 parts) so products accumulate near-exactly in
the fp32 PSUM accumulator; j (up to 1023) is split j = 256a + b so the j
rows stay exact in bf16.  K=15 rows; matmul cost only scales with N.

Pipeline per group: MMs -> one whole-tile PSUM->SBUF copy on a single
engine (alternating scalar/ACT and vector/DVE, each paired with its own
4-bank PSUM pool so WAR chains stay single-semaphore) -> one HWDGE DMA.
Single-engine copies keep every DMA at one wait and halve the program's
dependency-edge count: each edge costs a hardware event semaphore that the
NEFF epilogue resets ONE INSTRUCTION AT A TIME after the last DMA completes.

Structural notes:
 - built on bacc.Bacc, NOT raw bass.Bass, so that legalization runs;
 - every group gets its own SBUF output tile (10 MB of SBUF) so copies
   carry no WAR waits on earlier output DMAs;
 - all DMAs on the sync HWDGE path (gpsimd SWDGE stalls; scalar HWDGE
   hard-hung the device when tried for input loads).
"""

import sys
import types

import ml_dtypes
import numpy as np

import concourse.bacc as bacc
import concourse.bass as bass
import concourse.bass_utils as _bass_utils
import concourse.mybir as mybir
from concourse.bass_utils import run_bass_kernel_spmd
from concourse.tile import TileContext

# Cap walrus's event-semaphore pool.  The NEFF epilogue restores every
# ALLOCATED semaphore with an individual EVENT_SEMAPHORE instruction after
# the last DMA completes (~125 ns per sem per engine, all engines in
# lockstep): the default allocation (254 sems) costs ~8-9 us of pure tail
# overhead on every execution.  The kernel's live dependency window spans
# ~2 groups (~20 edges), so a 64-sem cap is comfortable.  Applied by
# appending the documented walrus_driver flag --max-sem-num to the compile
# invocation (an in-process compile option for THIS kernel's NEFF only).
_WALRUS_MAX_SEMS = "64"
_orig_run_command = _bass_utils.run_command


def _run_command_capped(argv, **kwargs):
    if (
        isinstance(argv, (list, tuple))
        and argv
        and "walrus_driver" in str(argv[0])
        and not any(str(a).startswith("--max-sem-num") for a in argv)
    ):
        argv = list(argv) + [f"--max-sem-num={_WALRUS_MAX_SEMS}"]
    return _orig_run_command(argv, **kwargs)


_bass_utils.run_command = _run_command_capped

# ---- problem constants (hardcoded; kernel.py must be self-contained) ----
N_PAIRS = 10_000_000
N_CORES = 8
CP = N_PAIRS // N_CORES  # 1,250,000 pairs per core
P = 128  # partitions
JSPAN = 1024  # pairs per partition per full group
GCOLS = 2 * JSPAN  # 2048 f32 per partition per full group
GPAIRS = P * JSPAN  # 131072 pairs per full group
NRAMP = 4  # leading [128,512] chunks (= one group-equivalent)
RSPAN = 256  # pairs per partition per ramp chunk
RPAIRS = P * RSPAN  # 32768 pairs per ramp chunk
NGF = CP // GPAIRS  # 9 full big groups (group 0 is the ramp)
TPAIRS = CP - NGF * GPAIRS  # 70352 tail pairs
TJSPAN = -(-TPAIRS // P)  # 550 pairs per partition in the tail group
TCOLS = 2 * TJSPAN  # 1100 f32 columns in the tail group
K = 15  # matmul contraction rows

# fp32-rounded constants, matching the reference's fp32 parameter rounding
DT = float(np.float32(0.01))
GDT_Y = float(np.float32(np.float32(-9.81) * np.float32(0.01)))  # fp32(g_y*dt)
C_Y = GDT_Y * DT / 2.0  # i^2 coefficient for y

_bf16 = ml_dtypes.bfloat16

# exposed for test.py introspection (exec_time_ns etc.)
LAST_RESULTS = None


def _ensure_axon_hooks_stub():
    """bass_utils imports antenv.axon_hooks when BASS_TRACE is set; some
    images lack that module.  Register a stub that degrades to the untraced
    path instead of crashing (test.py replaces it with a real NTFF hook)."""
    try:
        import antenv.axon_hooks  # noqa: F401

        return
    except ImportError:
        pass
    try:
        import antenv  # noqa: F401
    except ImportError:
        return
    stub = types.ModuleType("antenv.axon_hooks")
    stub.get_axon_ntff_profile_hook = lambda: None
    stub.set_axon_ntff_profile_hook = lambda h: None
    sys.modules["antenv.axon_hooks"] = stub


# host-side input packing: in0 gates the ramp (rh[:, :512] + ramp and tail
# lhsT); in1 carries the rest of rh and the big groups' lhsT.
HD0_COLS = 512 + (NRAMP + 1) * P  # rh[:, :512] + ramp lhsT + tail lhsT
HD1_COLS = (GCOLS - 512) + (NGF - 1) * P  # rh[:, 512:] + big-group lhsT


def _build_program() -> bass.Bass:
    # Bacc (not raw Bass): its finalize pipeline runs the sync-wait
    # legalization and register allocation walrus requires.
    nc = bacc.Bacc("TRN2", target_bir_lowering=False)
    # in0 carries what the ramp chunks and the tail group need, so one small
    # DMA gates the first matmul; the rest loads concurrently behind it.
    hd0 = nc.declare_dram_parameter(
        "hd0", [K, HD0_COLS], mybir.dt.bfloat16, isOutput=False
    )
    hd1 = nc.declare_dram_parameter(
        "hd1", [K, HD1_COLS], mybir.dt.bfloat16, isOutput=False
    )
    out = nc.declare_dram_parameter(
        "out", [NGF * P, GCOLS], mybir.dt.float32, isOutput=True
    )
    outt = nc.declare_dram_parameter(
        "outt", [P, TCOLS], mybir.dt.float32, isOutput=True
    )

    with TileContext(nc) as tc:
        with (
            tc.tile_pool(name="const", bufs=1) as cpool,
            tc.tile_pool(name="work", bufs=1) as wpool,
            tc.tile_pool(name="psum_a", bufs=1, space="PSUM") as ppool_a,
            tc.tile_pool(name="psum_b", bufs=1, space="PSUM") as ppool_b,
        ):
            in0_s = cpool.tile([K, HD0_COLS], mybir.dt.bfloat16)
            in1_s = cpool.tile([K, HD1_COLS], mybir.dt.bfloat16)
            nc.sync.dma_start(in0_s[:], hd0[:])
            nc.sync.dma_start(in1_s[:], hd1[:])

            def rh(c0, c1):
                # rh columns [0,512) live in in0; [512, GCOLS) in in1
                if c1 <= 512:
                    return in0_s[:, c0:c1]
                assert c0 >= 512
                return in1_s[:, c0 - 512 : c1 - 512]

            def lhsT(idx):
                # idx: 0..NRAMP-1 ramp, NRAMP tail, NRAMP+1.. big groups 1-9
                if idx <= NRAMP:
                    return in0_s[:, 512 + idx * P : 512 + (idx + 1) * P]
                i = idx - NRAMP - 1
                off = GCOLS - 512
                return in1_s[:, off + i * P : off + (i + 1) * P]

            pools = (ppool_a, ppool_b)
            copies = (nc.scalar.copy, nc.vector.tensor_copy)
            unit = 0  # alternator over (pool, copy engine)

            def produce(lt, cols, dst, name):
                nonlocal unit
                u = unit % 2
                unit += 1
                pt = pools[u].tile(
                    [P, GCOLS], mybir.dt.float32, name=f"pt{u}", tag=f"pt{u}"
                )
                for c0 in range(0, cols, 512):
                    c1 = min(c0 + 512, cols)
                    nc.tensor.matmul(
                        pt[:, c0:c1], lt, rh(c0, c1), start=True, stop=True
                    )
                ot = wpool.tile([P, cols], mybir.dt.float32, name=name, tag=name)
                copies[u](ot[:], pt[:, :cols])
                nc.sync.dma_start(dst, ot[:])

            # ramp chunks: pairs [c*RPAIRS, (c+1)*RPAIRS), 256-pair spans.
            # dst rows: out[32c:32c+32, :] viewed [32,4,512] -> partition
            # p = r*4+s owns the contiguous 2 KB at (r, s*512).
            for c in range(NRAMP):
                dst = out[c * 32 : (c + 1) * 32, :].rearrange(
                    "r (s q) -> (r s) q", s=4
                )
                produce(lhsT(c), 512, dst, f"or{c}")
            # tail group early (own contiguous tensor)
            produce(lhsT(NRAMP), TCOLS, outt[:], "ott")
            # big groups 1-9
            for g in range(1, NGF):
                produce(
                    lhsT(NRAMP + g), GCOLS, out[g * P : (g + 1) * P, :], f"og{g}"
                )
    nc.finalize()  # runs Bacc.compile(): reg alloc + sync-wait legalization
    return nc


def _split_bf16(x: np.ndarray, n: int):
    """Split x into n bf16 parts summing (nearly) exactly to x."""
    parts = []
    rem = np.asarray(x, dtype=np.float64).copy()
    for _ in range(n):
        p = rem.astype(_bf16)
        parts.append(p)
        rem = rem - p.astype(np.float64)
    return parts


def _host_tables(pos0: np.ndarray, vel0: np.ndarray):
    """Build per-core input tables (float64 math, cast at the end)."""
    ax, ay = float(pos0[0]), float(pos0[1])
    bx_c = DT * float(vel0[0])  # B_x (C_x = 0)
    by_c = DT * float(vel0[1]) - C_Y  # B_y

    # fixed rhs column patterns over ce in [0, GCOLS)
    ce = np.arange(GCOLS)
    j = (ce >> 1).astype(np.float64)
    odd = (ce & 1).astype(np.float64)
    even = 1.0 - odd
    ja = (256.0 * np.floor(j / 256.0)) * odd  # multiples of 256: exact bf16
    jb = (j - 256.0 * np.floor(j / 256.0)) * odd  # 0..255: exact bf16
    resid = np.where(ce & 1 == 1, C_Y * j * j, bx_c * j)
    r3 = _split_bf16(resid, 3)
    oddb = odd.astype(_bf16)
    evenb = even.astype(_bf16)
    rh_np = np.stack(
        [ja.astype(_bf16)] * 3
        + [jb.astype(_bf16)] * 3
        + r3
        + [oddb] * 3
        + [evenb] * 3
    )  # [K, GCOLS]

    def lt_block(q):  # q: [P] start pair index per partition
        s1_3 = _split_bf16(by_c + 2.0 * C_Y * q, 3)
        by3 = _split_bf16(ay + by_c * q + C_Y * q * q, 3)
        bx3 = _split_bf16(ax + bx_c * q, 3)
        ones = np.ones_like(s1_3[0])
        return np.stack(s1_3 + s1_3 + [ones] * 3 + by3 + bx3)  # [K, P]

    in_maps = []
    p_idx = np.arange(P, dtype=np.float64)
    for k in range(N_CORES):
        base = float(k * CP)
        blocks = []
        for c in range(NRAMP):  # ramp chunks
            blocks.append(lt_block(base + c * RPAIRS + p_idx * RSPAN))
        blocks.append(lt_block(base + NGF * GPAIRS + p_idx * TJSPAN))  # tail
        for g in range(1, NGF):  # big groups
            blocks.append(lt_block(base + g * GPAIRS + p_idx * JSPAN))
        lt_np = np.concatenate(blocks, axis=1)  # [K, 13*P]
        n_aux = (NRAMP + 1) * P
        in_maps.append(
            {
                "hd0": np.ascontiguousarray(
                    np.concatenate([rh_np[:, :512], lt_np[:, :n_aux]], axis=1)
                ),
                "hd1": np.ascontiguousarray(
                    np.concatenate([rh_np[:, 512:], lt_np[:, n_aux:]], axis=1)
                ),
            }
        )
    return in_maps


def kernel(ball_mass, ball_initial_position, ball_initial_velocity) -> np.ndarray:
    global LAST_RESULTS
    pos0 = np.asarray(ball_initial_position, dtype=np.float32)
    vel0 = np.asarray(ball_initial_velocity, dtype=np.float32)

    _ensure_axon_hooks_stub()
    nc = _build_program()
    in_maps = _host_tables(pos0, vel0)
    res = run_bass_kernel_spmd(nc, in_maps, core_ids=list(range(N_CORES)))
    LAST_RESULTS = res

    parts = []
    for r in res.results:
        arr = np.asarray(r["out"], dtype=np.float32)  # [NGF*P, GCOLS]
        tail = np.asarray(r["outt"], dtype=np.float32)  # [P, TCOLS]
        parts.append(arr.reshape(-1))  # ramp + big groups, contiguous
        parts.append(tail.reshape(-1)[: 2 * TPAIRS])
    return np.concatenate(parts).reshape(N_PAIRS, 2)


if __name__ == "__main__":
    import os

    pos0 = (
        np.load("/tmp/pos0.npy")
        if os.path.exists("/tmp/pos0.npy")
        else np.array([-1.866805, -0.25733662], np.float32)
    )
    vel0 = (
        np.load("/tmp/vel0.npy")
        if os.path.exists("/tmp/vel0.npy")
        else np.array([-0.847358, -1.5444987], np.float32)
    )
    outv = kernel(np.ones(()), pos0, vel0)
    i = np.arange(N_PAIRS, dtype=np.float64)[:, None]
    closed = (
        pos0.astype(np.float64)
        + i * DT * vel0.astype(np.float64)
        + np.array([0.0, GDT_Y * DT]) * i * (i - 1) / 2.0
    )
    err = np.abs(outv - closed)
    denom = np.maximum(np.abs(closed), 1e-12)
    print("closed-form maxabs-ratio rel err:", err.max() / np.abs(closed).max())
    print("closed-form max elementwise rel err:", (err / denom).max())
